# revision 33
# baseline (speedup 1.0000x reference)
"""ViTDet-style windowed attention w/ decomposed rel-pos, on 8 TRN2 NeuronCores.

Problem: x(8,32,32,768), 12 heads, hd=64, N=1024 tokens per image.
Sharding: pure data-parallel over B — core b handles image b; weights/tables
replicated; no collectives.

Per-core math (all matmuls bf16, fp32 PSUM accumulation):
  qkv^T[o, n]   = qkv_wT.T-chunks @ x^T          (o on partitions, n free)
  scores^T[j,i] = K'^T.T @ Q'^T   per head, where the 128-deep contraction is
                  [k(64) | onehot_jh(32) | onehot_jw(32)] x
                  [q_s(64) | rel_h^T(32) | rel_w^T(32)]
                  -> q.k + rel_h[i,jh] + rel_w[i,jw] in ONE matmul
  E = exp(scores^T)               (no max-subtraction: |scores| < 3)
  AV-transposed: out[i, v|1] accumulates matmul(lhsT=E-block, rhs=[v|1]) over
                 j-tiles — i lands on PARTITIONS, so the softmax denominator
                 (ones column) normalizes via per-partition tensor_scalar.
                 Streams V (65 cols) instead of E (512 cols): half the PE time
                 of the E-moving orientation, and no reciprocal-broadcast
                 matmuls.
  a^T blocks    = PE transpose of normalized attn rows (bf16 identity matmul),
                  c-blocks of 128 = head pairs -> proj contracts 128-deep.
  final^T = proj_wT.T @ a^T + b_eff,  b_eff = proj_w@b_v + proj_b (host)

Row-parity trick (phases 1-2): qkv^T psum tiles hold head 2t on partitions
0:64 and head 2t+1 on 64:128; DVE cannot shift partitions, so even heads keep
q/k on rows 0:64 (rel parts 64:128) while odd heads keep q/k on rows 64:128
(rel parts 0:64). The augmented inner product is row-permutation invariant
per head. Scores psum tiles are 2-bank [128,1024] so one exp covers two
matmuls.
"""

import numpy as np
import ml_dtypes

bf16 = ml_dtypes.bfloat16

B, H, W, C = 8, 32, 32, 768
NH, HD = 12, 64
N = H * W  # 1024
SCALE = HD ** -0.5
S_X = 16.0     # fp8 scale for x (max |x*16| ~ 82 < 240)
S_W = 128.0    # fp8 scale for qkv weights (keeps residuals out of subnormals)
S_XW = S_X * S_W

_NC = None


def _build(repeat=1):
    import concourse.mybir as mybir
    import concourse.tile as tile
    from concourse import bacc

    BF = mybir.dt.bfloat16
    F32 = mybir.dt.float32
    AF = mybir.ActivationFunctionType
    OP = mybir.AluOpType

    nc = bacc.Bacc(None, target_bir_lowering=False)

    FP8 = mybir.dt.float8e4
    xT_d = nc.declare_dram_parameter("xT", [128, 6, 1024], BF, isOutput=False)
    w8qk_d = nc.declare_dram_parameter("w8qk", [128, 6, 1536], FP8, isOutput=False)
    wr8qk_d = nc.declare_dram_parameter("wr8qk", [128, 6, 1536], FP8, isOutput=False)
    wv8_d = nc.declare_dram_parameter("wv8", [128, 6, 768], FP8, isOutput=False)
    wvr8_d = nc.declare_dram_parameter("wvr8", [128, 6, 768], FP8, isOutput=False)
    wproj_d = nc.declare_dram_parameter("wproj", [128, 6, 768], BF, isOutput=False)
    bqk_d = nc.declare_dram_parameter("bqk", [128, 12], F32, isOutput=False)
    bqk2_d = nc.declare_dram_parameter("bqk2", [128, 12], F32, isOutput=False)
    beff_d = nc.declare_dram_parameter("beff", [128, 6], F32, isOutput=False)
    rhT_d = nc.declare_dram_parameter("rhT", [64, 1024], BF, isOutput=False)
    rwT_d = nc.declare_dram_parameter("rwT", [64, 1024], BF, isOutput=False)
    eyeh_d = nc.declare_dram_parameter("eyeh", [32, 6, 1024], BF, isOutput=False)
    eyew_d = nc.declare_dram_parameter("eyew", [32, 6, 1024], BF, isOutput=False)
    eye128_d = nc.declare_dram_parameter("eye128", [128, 128], BF, isOutput=False)
    out_d = nc.declare_dram_parameter("out", [768, 1024], F32, isOutput=True)

    with tile.TileContext(nc) as tc, \
            tc.tile_pool(name="consts", bufs=1) as consts, \
            tc.tile_pool(name="epool", bufs=4) as epool, \
            tc.tile_pool(name="ps", bufs=2, space="PSUM") as pspool, \
            tc.tile_pool(name="sm", bufs=4, space="PSUM") as smpool, \
            tc.tile_pool(name="divp", bufs=4) as divp, \
            tc.tile_pool(name="outp", bufs=3) as outp:

        def emit():
            # xT shares its SBUF slot with aT (tag "big"): xT dies after
            # phase 3, aT (transposed attention) is born in phase 4.
            xT = consts.tile([128, 6, 1024], BF, tag="big")
            x2 = consts.tile([128, 6, 2, 1024], FP8)
            w8qk = consts.tile([128, 6, 1536], FP8, tag="w8proj")
            wr8qk = consts.tile([128, 6, 1536], FP8)
            wv8 = consts.tile([128, 6, 768], FP8)
            wvr8 = consts.tile([128, 6, 768], FP8)
            wproj = consts.tile([128, 6, 768], BF, tag="w8proj")
            bqk = consts.tile([128, 12], F32)
            bqk2 = consts.tile([128, 12], F32)
            beff = consts.tile([128, 6], F32)
            eye128 = consts.tile([128, 128], BF)
            # rel-pos tables duplicated on both partition halves so odd-parity
            # matmuls (operands on partitions 64:128) have an aligned lhsT.
            rhT = consts.tile([128, 1024], BF)
            rwT = consts.tile([128, 1024], BF)

            KT = consts.tile([128, 12, 1024], BF)   # augmented K'^T per head
            QT = consts.tile([128, 12, 1024], BF)   # augmented Q'^T per head
            Vb = consts.tile([128, 8, 12, 65], BF)  # [n-part, jt, head, v|1]
            # normalized attention, i on partitions: [i-part, it, (h, d)]
            attn = consts.tile([128, 8, 768], BF)

            def even_heads(ap):  # [p, 12, i] -> [p, 6, i] (heads 0,2,..)
                return ap.rearrange("p (hp two) i -> p two hp i", two=2)[:, 0, :, :]

            def odd_heads(ap):
                return ap.rearrange("p (hp two) i -> p two hp i", two=2)[:, 1, :, :]

            # DMA queues: SP HWDGE carries x + qkv weight chunks in need
            # order; Pool SWDGE carries the rel tables + one-hots; Act issues
            # NO DMAs before its x->fp8 quants (SEQ blocks per transfer).
            # x8 quant on Act + xr8 on DVE, per arriving (xT, W) ct-trio.
            for ct in range(6):
                nc.sync.dma_start(out=xT[:, ct, :], in_=xT_d[:, ct, :])
                nc.sync.dma_start(out=w8qk[:, ct, 0:768],
                                  in_=w8qk_d[:, ct, 0:768])
                nc.sync.dma_start(out=wr8qk[:, ct, 0:768],
                                  in_=wr8qk_d[:, ct, 0:768])
                nc.scalar.activation(
                    out=x2[:, ct, 0, :], in_=xT[:, ct, :],
                    func=AF.Copy, scale=S_X)
                nc.vector.scalar_tensor_tensor(
                    out=x2[:, ct, 1, :], in0=xT[:, ct, :], scalar=S_X,
                    in1=x2[:, ct, 0, :], op0=OP.mult, op1=OP.subtract)
            nc.sync.dma_start(out=bqk, in_=bqk_d[:])
            for ct in range(6):
                nc.sync.dma_start(out=w8qk[:, ct, 768:1536],
                                  in_=w8qk_d[:, ct, 768:1536])
                nc.sync.dma_start(out=wr8qk[:, ct, 768:1536],
                                  in_=wr8qk_d[:, ct, 768:1536])
            nc.gpsimd.dma_start(out=rhT[0:64, :], in_=rhT_d[:])
            nc.gpsimd.dma_start(out=rhT[64:128, :], in_=rhT_d[:])
            nc.gpsimd.dma_start(out=rwT[0:64, :], in_=rwT_d[:])
            nc.gpsimd.dma_start(out=rwT[64:128, :], in_=rwT_d[:])
            nc.gpsimd.dma_start(out=even_heads(KT[64:96]), in_=eyeh_d[:])
            nc.gpsimd.dma_start(out=odd_heads(KT[0:32]), in_=eyeh_d[:])
            nc.gpsimd.dma_start(out=even_heads(KT[96:128]), in_=eyew_d[:])
            nc.gpsimd.dma_start(out=odd_heads(KT[32:64]), in_=eyew_d[:])
            nc.sync.dma_start(out=beff, in_=beff_d[:])
            for ct in range(6):
                nc.sync.dma_start(out=wv8[:, ct, :], in_=wv8_d[:, ct, :])
                nc.sync.dma_start(out=wvr8[:, ct, :], in_=wvr8_d[:, ct, :])
            nc.sync.dma_start(out=eye128, in_=eye128_d[:])
            nc.sync.dma_start(out=wproj, in_=wproj_d[:])
            nc.scalar.dma_start(out=bqk2, in_=bqk2_d[:])
            # DVE "touch" of DMA-loaded scalars: absorbs the DMA-lane
            # semaphore waits so downstream tensor_scalar ops need only the
            # PE wait (fewer event-semaphore splits).
            tch = consts.tile([1, 18], F32)
            nc.vector.tensor_copy(out=tch[:, 0:12], in_=bqk[0:1, :])
            nc.vector.tensor_copy(out=tch[:, 12:18], in_=beff[0:1, :])

            nc.vector.memset(Vb[:, :, :, 64:65], 1.0)

            # ---- Per-pair producers (split for interleaving) ------------
            _p1ps = {}

            def p1_half(ot, ic):
                # qkv^T o-tile ic-half in its own 1-bank psum tile; on ic==1
                # repack both halves into QT/KT with the row-parity split.
                ps = smpool.tile([128, 512], F32, tag="sm", name=f"p1ps{ot}_{ic}")
                _p1ps[(ot, ic)] = ps
                osl = slice(ot * 128, (ot + 1) * 128)
                isl = slice(ic * 512, (ic + 1) * 512)
                DR = mybir.MatmulPerfMode.DoubleRow
                for ct in range(6):
                    # (x8|xr8) . (W8|W8): main + x-residual terms
                    nc.tensor.matmul(
                        ps,
                        w8qk[:, ct, None, osl].broadcast_to([128, 2, 128]),
                        x2[:, ct, :, isl],
                        start=(ct == 0), stop=False, perf_mode=DR,
                    )
                for cp in range(3):
                    # (x8_c|x8_c') . (Wr8_c|Wr8_c'): W-residual terms
                    nc.tensor.matmul(
                        ps,
                        wr8qk[:, 2 * cp:2 * cp + 2, osl],
                        x2[:, 2 * cp:2 * cp + 2, 0, isl],
                        start=False, stop=(cp == 2), perf_mode=DR,
                    )
                if ic != 1:
                    return
                on_act = ot in (0, 6)
                sc2 = (SCALE if ot < 6 else 1.0) / S_XW
                for i in range(2):
                    psh = _p1ps.pop((ot, i))
                    csl = slice(i * 512, (i + 1) * 512)
                    if ot < 6:
                        dsts = (QT[0:64, 2 * ot, csl],
                                QT[64:128, 2 * ot + 1, csl])
                    else:
                        h0 = (ot - 6) * 2
                        dsts = (KT[0:64, h0, csl], KT[64:128, h0 + 1, csl])
                    for rows, dst in zip((slice(0, 64), slice(64, 128)), dsts):
                        if on_act:
                            nc.scalar.activation(
                                out=dst, in_=psh[rows, :], func=AF.Identity,
                                scale=sc2, bias=bqk2[rows, ot:ot + 1])
                        else:
                            nc.vector.tensor_scalar(
                                out=dst, in0=psh[rows, :],
                                scalar1=bqk[rows, ot:ot + 1], scalar2=sc2,
                                op0=OP.add, op1=OP.mult)

            # rel_h^T / rel_w^T for heads (2t, 2t+1), four 1-bank psum
            # quarter-tiles (rel_h ih-halves natural-order, rel_w iw-major),
            # then 8 copies into QT's augmented rows (the rel_w copies
            # permute iw-major back to i-natural via strided APs).
            # Quadrant rows: 64:96 rel_h even | 96:128 rel_w even |
            # 0:32 rel_h odd | 32:64 rel_w odd.
            _p2ps = {}

            def p2_pair_mm(t, kind):
                qe = QT[0:64, 2 * t, :]
                qo = QT[64:128, 2 * t + 1, :]
                if kind == "h":
                    for half in range(2):
                        pr = smpool.tile([128, 512], F32, tag="sm",
                                         name=f"p2h{t}_{half}")
                        _p2ps[(t, "h", half)] = pr
                        for ihl in range(16):
                            ih = half * 16 + ihl
                            isl = slice(ih * 32, ih * 32 + 32)
                            lsl = slice(ihl * 32, ihl * 32 + 32)
                            nc.tensor.matmul(pr[64:96, lsl], rhT[0:64, isl],
                                             qe[:, isl], start=True, stop=True,
                                             tile_position=(0, 64))
                            nc.tensor.matmul(pr[0:32, lsl], rhT[64:128, isl],
                                             qo[:, isl], start=True, stop=True,
                                             tile_position=(64, 0))
                else:
                    qe_w = qe.rearrange("p (a b) -> p a b", b=32)
                    qo_w = qo.rearrange("p (a b) -> p a b", b=32)
                    for half in range(2):
                        pr = smpool.tile([128, 512], F32, tag="sm",
                                         name=f"p2w{t}_{half}")
                        _p2ps[(t, "w", half)] = pr
                        for iwl in range(16):
                            iw = half * 16 + iwl
                            wsl = slice(iw * 32, iw * 32 + 32)
                            lsl = slice(iwl * 32, iwl * 32 + 32)
                            nc.tensor.matmul(pr[96:128, lsl], rwT[0:64, wsl],
                                             qe_w[:, :, iw], start=True,
                                             stop=True, tile_position=(0, 96))
                            nc.tensor.matmul(pr[32:64, lsl], rwT[64:128, wsl],
                                             qo_w[:, :, iw], start=True,
                                             stop=True, tile_position=(64, 32))

            def p2_pair_copy(t):
                state = {"n": 0}
                def cp(out, in_):
                    state["n"] += 1
                    if t == 0 and state["n"] % 2 == 0:
                        nc.scalar.copy(out=out, in_=in_)
                    else:
                        nc.vector.tensor_copy(out=out, in_=in_)
                for half in range(2):
                    pr = _p2ps.pop((t, "h", half))
                    csl = slice(half * 512, (half + 1) * 512)
                    cp(QT[64:96, 2 * t, csl], pr[64:96, :])
                    cp(QT[0:32, 2 * t + 1, csl], pr[0:32, :])
                for half in range(2):
                    pr = _p2ps.pop((t, "w", half))
                    # pr cols are (iw-local 16, a 32); QT wants (a 32, iw 32)
                    src_e = pr[96:128, :].rearrange("p (b a) -> p a b", a=32)
                    src_o = pr[32:64, :].rearrange("p (b a) -> p a b", a=32)
                    dst_e = QT[96:128, 2 * t, :].rearrange(
                        "p (a b) -> p a b", b=32)[:, :, half * 16:(half + 1) * 16]
                    dst_o = QT[32:64, 2 * t + 1, :].rearrange(
                        "p (a b) -> p a b", b=32)[:, :, half * 16:(half + 1) * 16]
                    cp(dst_e, src_e)
                    cp(dst_o, src_o)

            def v_chunk(nt, ovc):
                pv = smpool.tile([128, 384], F32, tag="sm", name=f"pv{nt}_{ovc}")
                ntsl = slice(nt * 128, (nt + 1) * 128)
                vsl = slice(ovc * 384, (ovc + 1) * 384)
                DR = mybir.MatmulPerfMode.DoubleRow
                for ct in range(6):
                    nc.tensor.matmul(
                        pv,
                        x2[:, ct, :, ntsl],
                        wv8[:, ct, None, vsl].broadcast_to([128, 2, 384]),
                        start=(ct == 0), stop=False, perf_mode=DR,
                    )
                for cp in range(3):
                    nc.tensor.matmul(
                        pv,
                        x2[:, 2 * cp:2 * cp + 2, 0, ntsl],
                        wvr8[:, 2 * cp:2 * cp + 2, vsl],
                        start=False, stop=(cp == 2), perf_mode=DR,
                    )
                src = pv.rearrange("p (h d) -> p h d", d=64)
                nc.vector.tensor_scalar(
                    out=Vb[:, nt, ovc * 6:(ovc + 1) * 6, 0:64], in0=src,
                    scalar1=1.0 / S_XW, scalar2=None, op0=OP.mult)

            def head_av_half(h, half, E):
                # AV-T: per it-block of 128 queries, accumulate over jt:
                # out[i, 0:64]=sum_j E[j,i] v[j,:], out[i, 64]=rowsum.
                pa = smpool.tile([128, 4, 65], F32, tag="sm", name=f"pa{h}_{half}")
                for itl in range(4):
                    it = half * 4 + itl
                    for jt in range(8):
                        nc.tensor.matmul(
                            pa[:, itl, :],
                            E[:, jt, it * 128:(it + 1) * 128],
                            Vb[:, jt, h, 0:65],
                            start=(jt == 0), stop=(jt == 7))
                rec = divp.tile([128, 4], F32, tag="rec")
                with nc.allow_low_precision(reason="1/rowsum fine in f32"):
                    nc.vector.reciprocal(rec, pa[:, :, 64])
                nc.vector.tensor_tensor(
                    out=attn[:, half * 4:(half + 1) * 4,
                             h * 64:(h + 1) * 64],
                    in0=pa[:, :, 0:64],
                    in1=rec[:, :, None].broadcast_to([128, 4, 64]),
                    op=OP.mult)

            def transpose_half(t, half):
                # c-block t = heads (2t, 2t+1): aT[:, t, :] = attn-block^T
                for it in range(half * 4, half * 4 + 4):
                    tr = smpool.tile([128, 128], BF, tag="sm", name=f"tr{t}_{it}")
                    nc.tensor.transpose(
                        tr, attn[:, it, t * 128:(t + 1) * 128], eye128)
                    nc.vector.tensor_copy(
                        out=aT[:, t, it * 128:(it + 1) * 128], in_=tr)

            def transpose_pair(t):
                transpose_half(t, 0)
                transpose_half(t, 1)

            def proj_pair(cpair, ic):
                # proj for output-col tiles (2*cpair, 2*cpair+1), i-half ic.
                ps = pspool.tile([128, 1024], F32, tag="ps")
                for c2 in range(2):
                    cot = cpair * 2 + c2
                    for hp in range(6):
                        nc.tensor.matmul(ps[:, c2 * 512:(c2 + 1) * 512],
                                         wproj[:, hp, cot * 128:(cot + 1) * 128],
                                         aT[:, hp, ic * 512:(ic + 1) * 512],
                                         start=(hp == 0), stop=(hp == 5))
                for c2 in range(2):
                    cot = cpair * 2 + c2
                    osb = outp.tile([128, 512], F32, tag="osb")
                    nc.vector.tensor_scalar(
                        out=osb, in0=ps[:, c2 * 512:(c2 + 1) * 512],
                        scalar1=beff[:, cot:cot + 1], scalar2=None, op0=OP.add)
                    nc.sync.dma_start(
                        out=out_d[cot * 128:(cot + 1) * 128,
                                  ic * 512:(ic + 1) * 512],
                        in_=osb)

            aT = consts.tile([128, 6, 1024], BF, tag="big")

            # ---- Pipelined schedule ---------------------------------------
            # Steady state is Act(exp)-paced at ~1.04us per score psum tile;
            # PE filler work (next pair's P1/P2, v-chunks, AV-T, transposes)
            # is interleaved one item per score tile so PE never waits on the
            # psum ring.  Score tiles are ic-grouped (i-half 0 tiles first)
            # so AV halves start after 4 exps and the last pair's proj(ic=0)
            # overlaps the final ic=1 exps.
            def sc_tiles(h, E, ics=(0, 1)):
                for ic in ics:
                    for jtp in range(4):
                        def one(jtp=jtp, ic=ic, h=h, E=E):
                            ps = pspool.tile([128, 1024], F32, tag="ps")
                            for j2 in range(2):
                                jt = jtp * 2 + j2
                                nc.tensor.matmul(
                                    ps[:, j2 * 512:(j2 + 1) * 512],
                                    KT[:, h, jt * 128:(jt + 1) * 128],
                                    QT[:, h, ic * 512:(ic + 1) * 512],
                                    start=True, stop=True)
                            nc.scalar.activation(
                                out=E[:, jtp * 2:jtp * 2 + 2,
                                      ic * 512:(ic + 1) * 512],
                                in_=ps, func=AF.Exp)
                        yield one

            def p1_items(t):
                # per o-tile: ic-half accumulation groups + repacks
                for ot in (t, 6 + t):
                    for ic in range(2):
                        def half(ot=ot, ic=ic):
                            p1_half(ot, ic)
                        yield half

            def p2_items(t):
                yield lambda: p2_pair_mm(t, "h")
                yield lambda: p2_pair_mm(t, "w")
                yield lambda: p2_pair_copy(t)

            def av_items(h, E):
                for half in range(2):
                    def one(half=half, h=h, E=E):
                        head_av_half(h, half, E)
                    yield one

            def interleave(sc, fillers):
                sc = list(sc)
                fillers = list(fillers)
                fi = 0
                for i, s in enumerate(sc):
                    s()
                    tgt = (i + 1) * len(fillers) // len(sc)
                    while fi < tgt:
                        fillers[fi]()
                        fi += 1

            def chain(*gens):
                for g in gens:
                    yield from g

            # startup: pair 0 producers unoverlapped
            for f in chain(p1_items(0), p2_items(0)):
                f()
            E = {h: epool.tile([128, 8, 1024], BF, tag="E", name=f"E{h}")
                 for h in range(12)}
            vq = [(nt, ovc) for nt in range(8) for ovc in range(2)]
            V = lambda a, b: [(lambda x=x: v_chunk(*x)) for x in vq[a:b]]
            # iteration 0: heads 0,1 | fill: P1(1), P2(1), v[0:6]
            interleave(chain(sc_tiles(0, E[0]), sc_tiles(1, E[1])),
                       chain(p1_items(1), p2_items(1), V(0, 6)))
            # iteration 1: heads 2,3 | fill: P1(2), P2(2), v[6:16]
            interleave(chain(sc_tiles(2, E[2]), sc_tiles(3, E[3])),
                       chain(p1_items(2), p2_items(2), V(6, 16)))
            # iteration 2: heads 4,5 | fill: av(0,1) FIRST (they release the
            #              E-ring slots exp(4)/exp(5) wait on), then P1(3),
            #              P2(3).  NO transposes yet: the first aT write
            #              releases xT's shared SBUF slot, so it must come
            #              after the last xT reader (P1(5)).
            interleave(chain(sc_tiles(4, E[4]), sc_tiles(5, E[5])),
                       chain(av_items(0, E[0]), av_items(1, E[1]),
                             p1_items(3), p2_items(3)))
            # iteration 3: heads 6,7 | fill: P1(4), P2(4), av(2,3)
            interleave(chain(sc_tiles(6, E[6]), sc_tiles(7, E[7])),
                       chain(av_items(2, E[2]), av_items(3, E[3]),
                             p1_items(4), p2_items(4)))
            # iteration 4: heads 8,9 | fill: P1(5), P2(5), av(4,5), tp(0)
            interleave(chain(sc_tiles(8, E[8]), sc_tiles(9, E[9])),
                       chain(av_items(4, E[4]), av_items(5, E[5]),
                             p1_items(5), p2_items(5),
                             [lambda: transpose_pair(0)]))
            # iteration 5: heads 10,11 with ic0 tiles of BOTH heads first;
            # fill: av(6..9), tp(1..4)
            interleave(chain(sc_tiles(10, E[10], ics=(0,)),
                             sc_tiles(11, E[11], ics=(0,)),
                             sc_tiles(10, E[10], ics=(1,)),
                             sc_tiles(11, E[11], ics=(1,))),
                       chain(av_items(6, E[6]), av_items(7, E[7]),
                             [lambda: transpose_pair(1)],
                             av_items(8, E[8]), av_items(9, E[9]),
                             [lambda: transpose_pair(2)],
                             [lambda: transpose_pair(3)],
                             [lambda: transpose_pair(4)]))
            # tail: pair-5 ic0 AV/transpose + proj(ic=0) overlap the ic=1
            # exps still draining on Act; then the ic=1 remainder.
            head_av_half(10, 0, E[10])
            head_av_half(11, 0, E[11])
            transpose_half(5, 0)
            head_av_half(10, 1, E[10])
            proj_pair(0, 0)
            head_av_half(11, 1, E[11])
            proj_pair(1, 0)
            transpose_half(5, 1)
            proj_pair(2, 0)
            for cp in range(3):
                proj_pair(cp, 1)

        for _rep in range(repeat):
            emit()

    nc.compile()
    return nc


def _get_nc():
    global _NC
    if _NC is None:
        _NC = _build()
    return _NC


def _prep_inputs(x, qkv_w, qkv_b, proj_w, proj_b, rel_pos_h, rel_pos_w):
    x = np.asarray(x, np.float32)
    qkv_w = np.asarray(qkv_w, np.float32)
    qkv_b = np.asarray(qkv_b, np.float32)
    proj_w = np.asarray(proj_w, np.float32)
    proj_b = np.asarray(proj_b, np.float32)
    rel_pos_h = np.asarray(rel_pos_h, np.float32)
    rel_pos_w = np.asarray(rel_pos_w, np.float32)

    f8 = ml_dtypes.float8_e4m3
    wqkvT = np.ascontiguousarray(
        qkv_w.T.reshape(6, 128, 3 * C).transpose(1, 0, 2))  # [128, 6, 2304] f32
    w_s = wqkvT * S_W
    w8 = w_s.astype(f8)
    wr8 = (w_s - w8.astype(np.float32)).astype(f8)
    w8qk = np.ascontiguousarray(w8[:, :, 0:1536])
    wr8qk = np.ascontiguousarray(wr8[:, :, 0:1536])
    wv8 = np.ascontiguousarray(w8[:, :, 1536:2304])
    wvr8 = np.ascontiguousarray(wr8[:, :, 1536:2304])
    wproj = np.ascontiguousarray(
        proj_w.T.reshape(6, 128, C).transpose(1, 0, 2)).astype(bf16)
    bqk_raw = qkv_b[:2 * C].reshape(12, 128).T
    bqk = np.ascontiguousarray(bqk_raw * S_XW).astype(np.float32)
    bqk2 = np.ascontiguousarray(
        bqk_raw * np.r_[[SCALE] * 6, [1.0] * 6][None, :]).astype(np.float32)
    beff = np.ascontiguousarray(
        (proj_w @ qkv_b[2 * C:] + proj_b).reshape(6, 128).T).astype(np.float32)

    coords = np.arange(32)[:, None] - np.arange(32)[None, :] + 31
    rhT = np.ascontiguousarray(
        rel_pos_h[coords].transpose(2, 0, 1).reshape(64, 1024)).astype(bf16)
    rwT = np.ascontiguousarray(
        rel_pos_w[coords].transpose(2, 0, 1).reshape(64, 1024)).astype(bf16)

    base_h = np.kron(np.eye(32, dtype=np.float32), np.ones((1, 32), np.float32))
    base_w = np.tile(np.eye(32, dtype=np.float32), (1, 32))
    eyeh = np.ascontiguousarray(
        np.broadcast_to(base_h[:, None, :], (32, 6, 1024))).astype(bf16)
    eyew = np.ascontiguousarray(
        np.broadcast_to(base_w[:, None, :], (32, 6, 1024))).astype(bf16)
    eye128 = np.eye(128, dtype=np.float32).astype(bf16)

    shared = dict(w8qk=w8qk, wr8qk=wr8qk, wv8=wv8, wvr8=wvr8,
                  wproj=wproj, bqk=bqk, bqk2=bqk2, beff=beff,
                  rhT=rhT, rwT=rwT, eyeh=eyeh, eyew=eyew, eye128=eye128)
    in_maps = []
    for b in range(B):
        xT = np.ascontiguousarray(
            x[b].reshape(N, C).T.reshape(6, 128, N).transpose(1, 0, 2)
        ).astype(bf16)
        in_maps.append(dict(xT=xT, **shared))
    return in_maps


_last_results = None


def kernel(x, qkv_w, qkv_b, proj_w, proj_b, rel_pos_h, rel_pos_w):
    global _last_results
    from concourse.bass_utils import run_bass_kernel_spmd

    nc = _get_nc()
    in_maps = _prep_inputs(x, qkv_w, qkv_b, proj_w, proj_b,
                           rel_pos_h, rel_pos_w)
    res = run_bass_kernel_spmd(nc, in_maps, core_ids=list(range(8)))
    _last_results = res
    out = np.stack([
        np.asarray(res.results[b]["out"], np.float32).T.reshape(H, W, C)
        for b in range(B)
    ])
    return out
